# revision 9
# baseline (speedup 1.0000x reference)
"""Binarized linear + BatchNorm (eval) on 8 Trainium2 NeuronCores.

Computes: out = BN(sign(x) @ sign(weight).T)  for
  x [8192, 4096] f32, weight [4096, 4096] f32, BN vectors [4096] f32.

Strategy (v5)
-------------
Sharding: batch 4-way x out_features 2-way (8 cores). The kernel only
ever consumes sign(x) and sign(weight), so the host binarizes into fp8
(+/-1 and 0 are exact in fp8e4m3) and ships pre-tiled operands. The
device runs *only* the binary matmul (fp8 DoubleRow, K=256 per
instruction, fp32 PSUM accumulation -- bit-exact for +/-1 inputs) and
the folded-BN PSUM drain (a*acc + b on the vector engine, a/b computed
host-side), writing bf16 outputs. Per-core PE floor: 64 blocks x 16
matmuls x 512 cols @ 2.4 GHz ~ 221 us.

Layout: the host interleaves the 32 k-tiles as kt = 8g + 2*sm + r into
SBUF tiles [128, 16, 2, 2048] (dim1 = 4 chunks x {4 nb | 4 ot-groups},
dim2 = r, dim3 = sm*512 + col). This gives BOTH requirements at once:
  - the two k-rows a DoubleRow matmul reads sit 2048 B apart (adjacent
    layouts measured ~22% slower -- SBUF bank conflicts on the pair);
  - every DMA chunk is fully contiguous on DRAM AND SBUF side (strided
    SBUF writes measured ~25% slower on the HWDGE queues).

Schedule: the first four blocks run s-major interleaved across 4 PSUM
banks, chasing the k-quarter chunks of batch-tile 0 (sync queue) and
out-group 0 (scalar queue) as they land; W/X chunks are interleaved
across both queues in consumption-deadline order. Warm-up matmuls on
never-written scratch start the PE (and its activity clock -- stalls
re-throttle it to 4/8 for ~7 us) right after the engine preamble,
before any DMA lands. Outputs are staged 4 blocks/DMA on the sync
queue except the final group, which drains per-block to shorten the
tail.
"""

import numpy as np
from contextlib import ExitStack

B_FULL, IN, OUT = 8192, 4096, 4096
NB_CORES = 8
BI, OI = 4, 2            # batch x out_features core grid
BS = B_FULL // BI        # 2048 batch per core
OS = OUT // OI           # 2048 out_features per core
KT = IN // 128           # 32 k-tiles of 128
NS = KT // 2             # 16 k256 supertiles (DoubleRow)
OT = OS // 128           # 16 out tiles of 128
NBT = BS // 512          # 4 batch tiles of 512
BN_EPS = 1e-05
NWARM = 10               # warm-up matmuls before first real block

_CACHE = {}


def _build_program():
    import concourse.tile as tile
    from concourse import mybir, bacc

    F8 = mybir.dt.float8e4
    F32 = mybir.dt.float32
    BF16 = mybir.dt.bfloat16
    DR = mybir.MatmulPerfMode.DoubleRow

    nc = bacc.Bacc("TRN2", target_bir_lowering=False, debug=False,
                   num_devices=NB_CORES)
    # x6[nb, p, g, r, sm*512+q] = sign(x)[bi*BS + nb*512 + q,
    #                                     (8g + 2*sm + r)*128 + p]
    x6 = nc.declare_dram_parameter("x6", [NBT, 128, 4, 2, 2048], F8,
                                   isOutput=False)
    # w6[G, p, g, r, sm*512+q] = sign(weight)[oi*OS + G*512 + q,
    #                                         (8g + 2*sm + r)*128 + p]
    w6 = nc.declare_dram_parameter("w6", [4, 128, 4, 2, 2048], F8,
                                   isOutput=False)
    # ab[p, 0:OT] = a[ot*128 + p], ab[p, OT:2OT] = b[ot*128 + p]
    ab = nc.declare_dram_parameter("ab", [128, 2 * OT], F32, isOutput=False)
    # o[p, ot, nb, q] = out_core[ot*128 + p, nb*512 + q]
    o = nc.declare_dram_parameter("o", [128, OT, NBT, 512], BF16, isOutput=True)

    with tile.TileContext(nc) as tc:
        with ExitStack() as ctx:
            cons = ctx.enter_context(tc.tile_pool(name="cons", bufs=1))
            obp = ctx.enter_context(tc.tile_pool(name="ob", bufs=6))
            obs = ctx.enter_context(tc.tile_pool(name="obs", bufs=4))
            psp = ctx.enter_context(tc.tile_pool(name="ps", bufs=6, space="PSUM"))

            # warm-up scratch (memset by vector engine at t~0)
            sc_w = cons.tile([128, 2, 128], F8)
            sc_x = cons.tile([128, 2, 256], F8)
            nc.vector.memset(sc_w[:], 1.0)
            nc.vector.memset(sc_x[:], 1.0)

            # BN constants, one tiny DMA
            ab_sb = cons.tile([128, 2 * OT], F32)
            nc.gpsimd.dma_start(ab_sb[:], ab[:])

            # resident fp8 operands, permuted-kt layout (see docstring)
            xb = cons.tile([128, 16, 2, 2048], F8)   # dim1 = nb*4 + g
            wb = cons.tile([128, 16, 2, 2048], F8)   # dim1 = G*4 + g

            # DMA chunks in consumption-deadline order across both queues.
            # scalar: W group 0 in k-quarters (chased by the first block
            # group), then G1/G2 first halves, then batch tile 3
            for g in range(4):
                nc.scalar.dma_start(wb[:, 0 + g, :, :], w6[0, :, g])
            nc.scalar.dma_start(wb[:, 4:6, :, :], w6[1, :, 0:2])
            nc.scalar.dma_start(wb[:, 8:10, :, :], w6[2, :, 0:2])
            nc.scalar.dma_start(xb[:, 12:14, :, :], x6[3, :, 0:2])
            nc.scalar.dma_start(xb[:, 14:16, :, :], x6[3, :, 2:4])
            # sync: X batch tile 0 in k-quarters, G1/G2 second halves,
            # G3, batch tiles 1-2, then the batched outputs ride along
            for g in range(4):
                nc.sync.dma_start(xb[:, 0 + g, :, :], x6[0, :, g])
            nc.sync.dma_start(wb[:, 6:8, :, :], w6[1, :, 2:4])
            nc.sync.dma_start(wb[:, 10:12, :, :], w6[2, :, 2:4])
            nc.sync.dma_start(wb[:, 12:14, :, :], w6[3, :, 0:2])
            nc.sync.dma_start(wb[:, 14:16, :, :], w6[3, :, 2:4])
            for nb in (1, 2):
                nc.sync.dma_start(xb[:, nb * 4:nb * 4 + 2, :, :],
                                  x6[nb, :, 0:2])
                nc.sync.dma_start(xb[:, nb * 4 + 2:nb * 4 + 4, :, :],
                                  x6[nb, :, 2:4])

            # PE warm-up: starts the activity-clock ramp and absorbs the
            # preamble + first-chunk DMA latency
            warm = psp.tile([128, 512], F32, tag="warm", bufs=1, name="warm")
            for _ in range(NWARM):
                nc.tensor.matmul(warm[:, 0:256], sc_w[:], sc_x[:],
                                 start=True, stop=True, perf_mode=DR)

            def mm(acc, ot, nb, s):
                g, sm = divmod(s, 4)
                G, j = divmod(ot, 4)
                nc.tensor.matmul(
                    acc[:],
                    wb[:, G * 4 + g, :, sm * 512 + j * 128:
                       sm * 512 + (j + 1) * 128],
                    xb[:, nb * 4 + g, :, sm * 512:(sm + 1) * 512],
                    start=(s == 0), stop=(s == NS - 1),
                    perf_mode=DR)

            def drain(acc, ot, dst):
                nc.vector.tensor_scalar(
                    dst, acc[:],
                    ab_sb[:, ot:ot + 1], ab_sb[:, OT + ot:OT + ot + 1],
                    mybir.AluOpType.mult, mybir.AluOpType.add)

            for nb in range(NBT):
                for g4 in range(4):
                    last = (nb == NBT - 1 and g4 == 3)
                    if nb == 0 and g4 == 0:
                        # first 4 blocks s-major interleaved: the PE
                        # chases the X/W k-quarter chunks without gaps
                        accs = [psp.tile([128, 512], F32, tag="acc",
                                         name=f"acc_i{j}") for j in range(4)]
                        for s in range(NS):
                            for j in range(4):
                                mm(accs[j], j, 0, s)
                        ob4 = obp.tile([128, 4, 512], BF16, tag="ob",
                                       name="ob_0_0")
                        for j in range(4):
                            drain(accs[j], j, ob4[:, j, :])
                        nc.sync.dma_start(o[:, 0:4, 0, :], ob4[:])
                    elif not last:
                        ob4 = obp.tile([128, 4, 512], BF16, tag="ob",
                                       name=f"ob_{g4}_{nb}")
                        for j in range(4):
                            ot = 4 * g4 + j
                            acc = psp.tile([128, 512], F32, tag="acc",
                                           name=f"acc_{ot}_{nb}")
                            for s in range(NS):
                                mm(acc, ot, nb, s)
                            drain(acc, ot, ob4[:, j, :])
                        nc.sync.dma_start(o[:, 4 * g4:4 * g4 + 4, nb, :],
                                          ob4[:])
                    else:
                        # final group: per-block drain + DMA to keep the
                        # tail short
                        for j in range(4):
                            ot = 4 * g4 + j
                            acc = psp.tile([128, 512], F32, tag="acc",
                                           name=f"acc_{ot}_{nb}")
                            for s in range(NS):
                                mm(acc, ot, nb, s)
                            ob1 = obs.tile([128, 512], BF16, tag="obs",
                                           name=f"obs_{ot}")
                            drain(acc, ot, ob1[:])
                            nc.sync.dma_start(o[:, ot, nb, :], ob1[:])

    nc.compile()
    return nc


def make_in_maps(x, weight, bn_gamma, bn_beta, bn_mean, bn_var):
    import ml_dtypes
    f8 = ml_dtypes.float8_e4m3fn
    # host-side binarization: +/-1 (and 0) are exact in fp8e4m3
    xs8 = np.sign(x).astype(f8)
    ws8 = np.sign(weight).astype(f8)
    # [blk, q, g, sm, r, p] -> [blk, p, g, r, sm, q] -> [blk,128,4,2,2048]
    x6 = []
    for bi in range(BI):
        t = xs8[bi * BS:(bi + 1) * BS, :].reshape(NBT, 512, 4, 4, 2, 128)
        x6.append(np.ascontiguousarray(
            t.transpose(0, 5, 2, 4, 3, 1)).reshape(NBT, 128, 4, 2, 2048))
    w6 = []
    for oi in range(OI):
        t = ws8[oi * OS:(oi + 1) * OS, :].reshape(4, 512, 4, 4, 2, 128)
        w6.append(np.ascontiguousarray(
            t.transpose(0, 5, 2, 4, 3, 1)).reshape(4, 128, 4, 2, 2048))
    # folded BN: out = a*acc + b
    a = (bn_gamma / np.sqrt(bn_var + BN_EPS)).astype(np.float32)
    b = (bn_beta - bn_mean * a).astype(np.float32)
    ab = []
    for oi in range(OI):
        sl = slice(oi * OS, (oi + 1) * OS)
        ab.append(np.ascontiguousarray(np.concatenate(
            [a[sl].reshape(OT, 128).T, b[sl].reshape(OT, 128).T], axis=1)))
    in_maps = []
    for c in range(NB_CORES):
        bi, oi = divmod(c, OI)
        in_maps.append({"x6": x6[bi], "w6": w6[oi], "ab": ab[oi]})
    return in_maps


def kernel(x, weight, bn_gamma, bn_beta, bn_mean, bn_var):
    from concourse.bass_utils import run_bass_kernel_spmd

    x = np.asarray(x, dtype=np.float32)
    weight = np.asarray(weight, dtype=np.float32)
    bn_gamma = np.asarray(bn_gamma, dtype=np.float32)
    bn_beta = np.asarray(bn_beta, dtype=np.float32)
    bn_mean = np.asarray(bn_mean, dtype=np.float32)
    bn_var = np.asarray(bn_var, dtype=np.float32)

    if "nc" not in _CACHE:
        _CACHE["nc"] = _build_program()
    nc = _CACHE["nc"]

    in_maps = make_in_maps(x, weight, bn_gamma, bn_beta, bn_mean, bn_var)

    res = run_bass_kernel_spmd(nc, in_maps, list(range(NB_CORES)))
    _CACHE["last_results"] = res

    out = np.empty((B_FULL, OUT), dtype=np.float32)
    for c in range(NB_CORES):
        bi, oi = divmod(c, OI)
        # o[p, ot, nb, q] -> out[nb*512+q (batch), ot*128+p (feature)]
        oc = np.asarray(res.results[c]["o"]).astype(np.float32)
        oc = oc.transpose(2, 3, 1, 0).reshape(BS, OS)
        out[bi * BS:(bi + 1) * BS, oi * OS:(oi + 1) * OS] = oc
    return out


# revision 11
# speedup vs baseline: 1.1861x; 1.1861x over previous
"""Binarized linear + BatchNorm (eval) on 8 Trainium2 NeuronCores.

Computes: out = BN(sign(x) @ sign(weight).T)  for
  x [8192, 4096] f32, weight [4096, 4096] f32, BN vectors [4096] f32.

Strategy (v5)
-------------
Sharding: batch 4-way x out_features 2-way (8 cores). The kernel only
ever consumes sign(x) and sign(weight), so the host binarizes into fp8
(+/-1 and 0 are exact in fp8e4m3) and ships pre-tiled operands. The
device runs *only* the binary matmul (fp8 DoubleRow, K=256 per
instruction, fp32 PSUM accumulation -- bit-exact for +/-1 inputs) and
the folded-BN PSUM drain (a*acc + b on the vector engine, a/b computed
host-side), writing bf16 outputs. Per-core PE floor: 64 blocks x 16
matmuls x 512 cols @ 2.4 GHz ~ 221 us.

Layout: the host interleaves the 32 k-tiles as kt = 8g + 2*sm + r into
SBUF tiles [128, 16, 2, 2048] (dim1 = 4 chunks x {4 nb | 4 ot-groups},
dim2 = r, dim3 = sm*512 + col). This gives BOTH requirements at once:
  - the two k-rows a DoubleRow matmul reads sit 2048 B apart (adjacent
    layouts measured ~22% slower -- SBUF bank conflicts on the pair);
  - every DMA chunk is fully contiguous on DRAM AND SBUF side (strided
    SBUF writes measured ~25% slower on the HWDGE queues).

Schedule: the first four blocks run s-major interleaved across 4 PSUM
banks, chasing the k-quarter chunks of batch-tile 0 (sync queue) and
out-group 0 (scalar queue) as they land; W/X chunks are interleaved
across both queues in consumption-deadline order. Warm-up matmuls on
never-written scratch start the PE (and its activity clock -- stalls
re-throttle it to 4/8 for ~7 us) right after the engine preamble,
before any DMA lands. Outputs are staged 4 blocks/DMA on the sync
queue except the final group, which drains per-block to shorten the
tail.
"""

import numpy as np
from contextlib import ExitStack

B_FULL, IN, OUT = 8192, 4096, 4096
NB_CORES = 8
BI, OI = 4, 2            # batch x out_features core grid
BS = B_FULL // BI        # 2048 batch per core
OS = OUT // OI           # 2048 out_features per core
KT = IN // 128           # 32 k-tiles of 128
NS = KT // 2             # 16 k256 supertiles (DoubleRow)
OT = OS // 128           # 16 out tiles of 128
NBT = BS // 512          # 4 batch tiles of 512
BN_EPS = 1e-05
NWARM = 10               # warm-up matmuls before first real block

_CACHE = {}


def _build_program():
    import concourse.tile as tile
    from concourse import mybir, bacc

    F8 = mybir.dt.float8e4
    F32 = mybir.dt.float32
    BF16 = mybir.dt.bfloat16
    DR = mybir.MatmulPerfMode.DoubleRow

    nc = bacc.Bacc("TRN2", target_bir_lowering=False, debug=False,
                   num_devices=NB_CORES)
    # x6[nb, p, g, r, sm*512+q] = sign(x)[bi*BS + nb*512 + q,
    #                                     (8g + 2*sm + r)*128 + p]
    x6 = nc.declare_dram_parameter("x6", [NBT, 128, 4, 2, 2048], F8,
                                   isOutput=False)
    # w6[G, p, g, r, sm*512+q] = sign(weight)[oi*OS + G*512 + q,
    #                                         (8g + 2*sm + r)*128 + p]
    w6 = nc.declare_dram_parameter("w6", [4, 128, 4, 2, 2048], F8,
                                   isOutput=False)
    # ab[p, 0:OT] = a[ot*128 + p], ab[p, OT:2OT] = b[ot*128 + p]
    ab = nc.declare_dram_parameter("ab", [128, 2 * OT], F32, isOutput=False)
    # o[p, ot, nb, q] = out_core[ot*128 + p, nb*512 + q]
    o = nc.declare_dram_parameter("o", [128, OT, NBT, 512], BF16, isOutput=True)

    with tile.TileContext(nc) as tc:
        with ExitStack() as ctx:
            cons = ctx.enter_context(tc.tile_pool(name="cons", bufs=1))
            obp = ctx.enter_context(tc.tile_pool(name="ob", bufs=6))
            obs = ctx.enter_context(tc.tile_pool(name="obs", bufs=4))
            psp = ctx.enter_context(tc.tile_pool(name="ps", bufs=6, space="PSUM"))

            # warm-up scratch (memset by vector engine at t~0)
            sc_w = cons.tile([128, 2, 128], F8)
            sc_x = cons.tile([128, 2, 256], F8)
            nc.vector.memset(sc_w[:], 1.0)
            nc.vector.memset(sc_x[:], 1.0)

            # BN constants, one tiny DMA
            ab_sb = cons.tile([128, 2 * OT], F32)
            nc.gpsimd.dma_start(ab_sb[:], ab[:])

            # resident fp8 operands, permuted-kt layout (see docstring)
            xb = cons.tile([128, 16, 2, 2048], F8)   # dim1 = nb*4 + g
            wb = cons.tile([128, 16, 2, 2048], F8)   # dim1 = G*4 + g

            # DMA chunks in consumption-deadline order across both queues.
            # scalar: W group 0 in k-quarters (chased by the first block
            # group), then G1/G2 first halves, then batch tile 3
            for g in range(4):
                nc.scalar.dma_start(wb[:, 0 + g, :, :], w6[0, :, g])
            nc.scalar.dma_start(wb[:, 4:6, :, :], w6[1, :, 0:2])
            nc.scalar.dma_start(wb[:, 8:10, :, :], w6[2, :, 0:2])
            nc.scalar.dma_start(xb[:, 12:14, :, :], x6[3, :, 0:2])
            nc.scalar.dma_start(xb[:, 14:16, :, :], x6[3, :, 2:4])
            # sync: X batch tile 0 in k-quarters, G1/G2 second halves,
            # G3, batch tiles 1-2, then the batched outputs ride along
            for g in range(4):
                nc.sync.dma_start(xb[:, 0 + g, :, :], x6[0, :, g])
            nc.sync.dma_start(wb[:, 6:8, :, :], w6[1, :, 2:4])
            nc.sync.dma_start(wb[:, 10:12, :, :], w6[2, :, 2:4])
            nc.sync.dma_start(wb[:, 12:14, :, :], w6[3, :, 0:2])
            nc.sync.dma_start(wb[:, 14:16, :, :], w6[3, :, 2:4])
            for nb in (1, 2):
                nc.sync.dma_start(xb[:, nb * 4:nb * 4 + 2, :, :],
                                  x6[nb, :, 0:2])
                nc.sync.dma_start(xb[:, nb * 4 + 2:nb * 4 + 4, :, :],
                                  x6[nb, :, 2:4])

            # PE warm-up: starts the activity-clock ramp and absorbs the
            # preamble + first-chunk DMA latency
            warm = psp.tile([128, 512], F32, tag="warm", bufs=1, name="warm")
            for _ in range(NWARM):
                nc.tensor.matmul(warm[:, 0:256], sc_w[:], sc_x[:],
                                 start=True, stop=True, perf_mode=DR)

            def mm(acc, ot, nb, s):
                g, sm = divmod(s, 4)
                G, j = divmod(ot, 4)
                # W's sm phase is stored shifted by 2 (1024 B) so the
                # concurrent X-stream and LDWEIGHTS fetches never share
                # an address phase mod 2048 (else ~20% slower matmuls)
                smw = (sm + 2) % 4
                nc.tensor.matmul(
                    acc[:],
                    wb[:, G * 4 + g, :, smw * 512 + j * 128:
                       smw * 512 + (j + 1) * 128],
                    xb[:, nb * 4 + g, :, sm * 512:(sm + 1) * 512],
                    start=(s == 0), stop=(s == NS - 1),
                    perf_mode=DR)

            def drain(acc, ot, dst):
                nc.vector.tensor_scalar(
                    dst, acc[:],
                    ab_sb[:, ot:ot + 1], ab_sb[:, OT + ot:OT + ot + 1],
                    mybir.AluOpType.mult, mybir.AluOpType.add)

            for nb in range(NBT):
                for g4 in range(4):
                    last = (nb == NBT - 1 and g4 == 3)
                    if nb == 0 and g4 == 0:
                        # first 4 blocks s-major interleaved: the PE
                        # chases the X/W k-quarter chunks without gaps
                        accs = [psp.tile([128, 512], F32, tag="acc",
                                         name=f"acc_i{j}") for j in range(4)]
                        for s in range(NS):
                            for j in range(4):
                                mm(accs[j], j, 0, s)
                        ob4 = obp.tile([128, 4, 512], BF16, tag="ob",
                                       name="ob_0_0")
                        for j in range(4):
                            drain(accs[j], j, ob4[:, j, :])
                        nc.sync.dma_start(o[:, 0:4, 0, :], ob4[:])
                    elif not last:
                        ob4 = obp.tile([128, 4, 512], BF16, tag="ob",
                                       name=f"ob_{g4}_{nb}")
                        for j in range(4):
                            ot = 4 * g4 + j
                            acc = psp.tile([128, 512], F32, tag="acc",
                                           name=f"acc_{ot}_{nb}")
                            for s in range(NS):
                                mm(acc, ot, nb, s)
                            drain(acc, ot, ob4[:, j, :])
                        nc.sync.dma_start(o[:, 4 * g4:4 * g4 + 4, nb, :],
                                          ob4[:])
                    else:
                        # final group: per-block drain + DMA to keep the
                        # tail short
                        for j in range(4):
                            ot = 4 * g4 + j
                            acc = psp.tile([128, 512], F32, tag="acc",
                                           name=f"acc_{ot}_{nb}")
                            for s in range(NS):
                                mm(acc, ot, nb, s)
                            ob1 = obs.tile([128, 512], BF16, tag="obs",
                                           name=f"obs_{ot}")
                            drain(acc, ot, ob1[:])
                            nc.sync.dma_start(o[:, ot, nb, :], ob1[:])

    nc.compile()
    return nc


def make_in_maps(x, weight, bn_gamma, bn_beta, bn_mean, bn_var):
    import ml_dtypes
    f8 = ml_dtypes.float8_e4m3fn
    # host-side binarization: +/-1 (and 0) are exact in fp8e4m3
    xs8 = np.sign(x).astype(f8)
    ws8 = np.sign(weight).astype(f8)
    # [blk, q, g, sm, r, p] -> [blk, p, g, r, sm, q] -> [blk,128,4,2,2048]
    x6 = []
    for bi in range(BI):
        t = xs8[bi * BS:(bi + 1) * BS, :].reshape(NBT, 512, 4, 4, 2, 128)
        x6.append(np.ascontiguousarray(
            t.transpose(0, 5, 2, 4, 3, 1)).reshape(NBT, 128, 4, 2, 2048))
    w6 = []
    for oi in range(OI):
        t = ws8[oi * OS:(oi + 1) * OS, :].reshape(4, 512, 4, 4, 2, 128)
        t = t.transpose(0, 5, 2, 4, 3, 1)      # [G, p, g, r, sm, q]
        t = t[:, :, :, :, [2, 3, 0, 1], :]     # store sm shifted by 2
        w6.append(np.ascontiguousarray(t).reshape(4, 128, 4, 2, 2048))
    # folded BN: out = a*acc + b
    a = (bn_gamma / np.sqrt(bn_var + BN_EPS)).astype(np.float32)
    b = (bn_beta - bn_mean * a).astype(np.float32)
    ab = []
    for oi in range(OI):
        sl = slice(oi * OS, (oi + 1) * OS)
        ab.append(np.ascontiguousarray(np.concatenate(
            [a[sl].reshape(OT, 128).T, b[sl].reshape(OT, 128).T], axis=1)))
    in_maps = []
    for c in range(NB_CORES):
        bi, oi = divmod(c, OI)
        in_maps.append({"x6": x6[bi], "w6": w6[oi], "ab": ab[oi]})
    return in_maps


def kernel(x, weight, bn_gamma, bn_beta, bn_mean, bn_var):
    from concourse.bass_utils import run_bass_kernel_spmd

    x = np.asarray(x, dtype=np.float32)
    weight = np.asarray(weight, dtype=np.float32)
    bn_gamma = np.asarray(bn_gamma, dtype=np.float32)
    bn_beta = np.asarray(bn_beta, dtype=np.float32)
    bn_mean = np.asarray(bn_mean, dtype=np.float32)
    bn_var = np.asarray(bn_var, dtype=np.float32)

    if "nc" not in _CACHE:
        _CACHE["nc"] = _build_program()
    nc = _CACHE["nc"]

    in_maps = make_in_maps(x, weight, bn_gamma, bn_beta, bn_mean, bn_var)

    res = run_bass_kernel_spmd(nc, in_maps, list(range(NB_CORES)))
    _CACHE["last_results"] = res

    out = np.empty((B_FULL, OUT), dtype=np.float32)
    for c in range(NB_CORES):
        bi, oi = divmod(c, OI)
        # o[p, ot, nb, q] -> out[nb*512+q (batch), ot*128+p (feature)]
        oc = np.asarray(res.results[c]["o"]).astype(np.float32)
        oc = oc.transpose(2, 3, 1, 0).reshape(BS, OS)
        out[bi * BS:(bi + 1) * BS, oi * OS:(oi + 1) * OS] = oc
    return out
